# revision 12
# baseline (speedup 1.0000x reference)
"""Chamfer loss kernel for Trainium2 (8 NeuronCores, data-parallel over batch).

Problem: B=8, N=M=4096, D=3 fp32 point clouds.
  loss = mean_b mean_n min_m ||p_bn - g_bm||^2  +  mean_b mean_m min_n ||.||^2

Per-core structure (one batch element per core):
  - K=7 f32r matmuls produce distance tiles d[n, m] in PSUM (row pairing
    lhsT row k x rhs row k): k0,k1: 1*(|g|^2 hi/lo); k2,k3: (|p|^2 hi/lo)*1;
    k4-6: (-2 p_d)*g_d. The operand strips are fully precomputed on the HOST
    (f32r == round-to-nearest-even at 11 mantissa bits, verified on HW), so
    on-chip prep is just 2 HBM loads + 6 partition-shift DMAs replicating the
    strips to the 4 PE row-group bases (tile_position packing: 4 concurrent
    matmuls per [128, 2048] span).
  - Engine split (measured costs / span of [128,2048]):
      ACT: PSUM->SBUF bf16 extraction, 1.96us/span, 30 of 32 tiles.
      DVE: all reductions: rowmin via bf16 fold-trees batched over 8-tile
           rings (TT min at 2x mode, fold 4096->128, then one 1x
           tensor_reduce), and colacc via quad-batched TT min chains.
      Tiles 0-1 use the DVE fused tensor_scalar(max0, accum_out=min) path
      reading PSUM directly (extraction+rowmin in one 1x op) so DVE starts
      ~8us earlier while ACT warms up.
  - Tail: colacc [128, 4096] partition-min via PE transpose (bf16 identity)
    + 4 pipelined free-axis reduces; clamp minima at 0; final sums via
    ones-matmul; host averages the 8 per-core scalars.
"""

import sys

import numpy as np

sys.path.insert(0, "/opt/trn_rl_repo")

import bass_rust
import concourse.bass as bass
import concourse.mybir as mybir
from concourse.bass_utils import run_bass_kernel_spmd
from concourse.masks import make_identity
from concourse.tile import TileContext

B, N, M, D = 8, 4096, 4096, 3
NT = N // 128  # 32 n-tiles
K = 7
F32 = mybir.dt.float32
F32R = mybir.dt.float32r
BF16 = mybir.dt.bfloat16
BIG = 3.0e38

# ---------------------------------------------------------------------------
# walrus in this container rejects >1 sync-wait per instruction; spill the
# extras onto engine-matched NoOps placed immediately before the instruction.
_nop_counter = [0]


def _split_multi_waits(nc):
    for func in nc.m.functions:
        for bb in func.blocks:
            out = []
            dirty = False
            for inst in bb.instructions:
                si = inst.sync_info
                if si is not None and len(si.on_wait) > 1:
                    waits = list(si.on_wait)
                    for w in waits[:-1]:
                        _nop_counter[0] += 1
                        nop = mybir.InstNoOp(
                            name=f"I-waitsplit-{_nop_counter[0]}", ins=[], outs=[]
                        )
                        nop.engine = inst.engine
                        nop.sync_info = bass_rust.SyncInfo(on_wait=[w], on_update=[])
                        out.append(nop)
                    inst.sync_info = bass_rust.SyncInfo(
                        on_wait=[waits[-1]], on_update=list(si.on_update)
                    )
                    dirty = True
                out.append(inst)
            if dirty:
                bb.instructions = out
    return nc


# ---------------------------------------------------------------------------


def build_nc():
    nc = bass.Bass("TRN2")
    zp_d = nc.dram_tensor("zp", [K, N], F32, kind="ExternalInput")
    zg_d = nc.dram_tensor("zg", [K, M], F32, kind="ExternalInput")
    out_d = nc.dram_tensor("out", [1, 1], F32, kind="ExternalOutput")

    with TileContext(nc) as tc:
        with (
            tc.tile_pool(name="persist", bufs=1) as persist,
            tc.tile_pool(name="dsbp", bufs=2) as dsbp,
            tc.tile_pool(name="ffp", bufs=1) as ffp,
        ):
            # --- prep: load strips fp32 via fast HWDGE, cast to f32r on
            # DVE (zp) + ACT (zg) in parallel, replicate to the 4 PE
            # row-group bases via sync/scalar HWDGE queues (gpsimd SWDGE is
            # slow and its drains can block the PE). ---
            zp_f = persist.tile([K, N], F32)
            zg_f = persist.tile([K, M], F32)
            nc.sync.dma_start(out=zp_f, in_=zp_d.ap())
            nc.scalar.dma_start(out=zg_f, in_=zg_d.ap())
            zp_pk = persist.tile([96 + K, N], F32R)
            zg_pk = persist.tile([96 + K, M], F32R)
            nc.vector.tensor_copy(zp_pk[0:K, :], zp_f)
            nc.scalar.copy(zg_pk[0:K, :], zg_f)
            qs = [nc.sync, nc.scalar]
            for g in range(1, 4):
                qs[g % 2].dma_start(
                    out=zp_pk[32 * g : 32 * g + K, :], in_=zp_pk[0:K, :]
                )
                qs[(g + 1) % 2].dma_start(
                    out=zg_pk[32 * g : 32 * g + K, :], in_=zg_pk[0:K, :]
                )
            # identity for tail transposes (off critical path)
            ident16 = persist.tile([128, 128], BF16)
            nc.vector.memset(ident16, 0.0)
            make_identity(nc, ident16, nomemset=True)

            colacc = persist.tile([128, M], BF16)
            rowmins = persist.tile([128, NT], F32)
            rowspan = persist.tile([128, 4], F32)

            psp = tc.alloc_tile_pool(name="psum_all", bufs=2, space="PSUM")

            def mm_span(t, h):
                """4 row-group-packed concurrent matmuls for n-tile t,
                half-span h -> returns the [128, 2048] PSUM tile."""
                ps = psp.tile([128, 2048], F32, name="ps_main", tag="ps_main")
                for g in range(4):
                    col0 = 2048 * h + 512 * g
                    nc.tensor.matmul(
                        ps[:, 512 * g : 512 * (g + 1)],
                        zp_pk[32 * g : 32 * g + K, 128 * t : 128 * (t + 1)],
                        zg_pk[32 * g : 32 * g + K, col0 : col0 + 512],
                        start=True,
                        stop=True,
                        tile_position=(32 * g, 0),
                    )
                return ps

            def extract(t, dsb, slot):
                for h in range(2):
                    ps = mm_span(t, h)
                    nc.scalar.copy(
                        out=dsb[:, slot, 2048 * h : 2048 * (h + 1)], in_=ps
                    )

            # --- tiles 0, 1: DVE fused extraction (tensor_scalar from PSUM:
            # out = relu(d) -> bf16, accum_out = rowmin of span), span-
            # interleaved with ACT extraction of tiles 2, 3 so both engines
            # start working ~immediately. ---
            dsb23 = dsbp.tile([128, 4, M], BF16, name="dsb", tag="dsb")
            d1 = dsb23[:, 3, :]
            for t in range(2):
                for h in range(2):
                    ps = mm_span(t, h)
                    dest = colacc if t == 0 else d1
                    nc.vector.tensor_scalar(
                        out=dest[:, 2048 * h : 2048 * (h + 1)],
                        in0=ps,
                        scalar1=0.0,
                        scalar2=BIG,
                        op0=mybir.AluOpType.max,
                        op1=mybir.AluOpType.min,
                        accum_out=rowspan[:, 2 * t + h : 2 * t + h + 1],
                    )
                    ps2 = mm_span(t + 2, h)
                    nc.scalar.copy(
                        out=dsb23[:, t, 2048 * h : 2048 * (h + 1)], in_=ps2
                    )
            nc.vector.tensor_tensor(
                out=colacc, in0=d1, in1=colacc, op=mybir.AluOpType.min
            )
            # rowmins[:, 0:2] = pairwise min of the 4 span minima
            nc.vector.tensor_tensor(
                out=rowmins[:, 0:2],
                in0=rowspan.rearrange("p (t h) -> p t h", h=2)[:, :, 0],
                in1=rowspan.rearrange("p (t h) -> p t h", h=2)[:, :, 1],
                op=mybir.AluOpType.min,
            )

            def colacc_group(dsb, n):
                # elementwise-min of n extracted tiles into colacc
                if n == 4:
                    nc.vector.tensor_tensor(
                        out=dsb[:, 0:2, :], in0=dsb[:, 0:2, :],
                        in1=dsb[:, 2:4, :], op=mybir.AluOpType.min,
                    )
                r = dsb[:, 0, :]
                nc.vector.tensor_tensor(
                    out=r, in0=dsb[:, 0, :], in1=dsb[:, 1, :],
                    op=mybir.AluOpType.min,
                )
                nc.vector.tensor_tensor(
                    out=colacc, in0=r, in1=colacc, op=mybir.AluOpType.min
                )

            def fold_l1(dsb, n, ff1, fill):
                # level-1 fold of n tiles into ff1[:, fill : fill+n, :]
                v = dsb.rearrange("p i (l m) -> p i l m", l=2)[:, 0:n]
                nc.vector.tensor_tensor(
                    out=ff1[:, fill : fill + n, :],
                    in0=v[:, :, 0, :],
                    in1=v[:, :, 1, :],
                    op=mybir.AluOpType.min,
                )

            def fold_tail(ff1, n, t0):
                # fold n packed tiles 2048 -> 128, then one batched reduce
                f = ff1
                w = 2048
                for name in ("ff2", "ff3", "ff4", "ff5"):
                    nf = ffp.tile([128, 8, w // 2], BF16, name=name, tag=name)
                    v = f.rearrange("p i (l m) -> p i l m", l=2)[:, 0:n]
                    nc.vector.tensor_tensor(
                        out=nf[:, 0:n, :], in0=v[:, :, 0, :], in1=v[:, :, 1, :],
                        op=mybir.AluOpType.min,
                    )
                    f = nf
                    w //= 2
                nc.vector.tensor_reduce(
                    out=rowmins[:, t0 : t0 + n],
                    in_=f[:, 0:n, :],
                    axis=mybir.AxisListType.X,
                    op=mybir.AluOpType.min,
                )

            # group plan over tiles 2..31: pairs {2,3} {4,5}, quads {6-9}..
            # {26-29}, pair {30,31}. Early pairs keep DVE fed while ACT
            # builds its lead; the fold ring accumulates L1 folds of
            # consecutive tiles and flushes a batched fold-tail at 8.
            groups = [(2, 2), (4, 2)] + [(6 + 4 * i, 4) for i in range(6)] + [(30, 2)]
            ring = None  # (ff1, t_start, fill)
            for gi, (t0, n) in enumerate(groups):
                if gi == 0:
                    dsb = dsb23  # already extracted (interleaved above)
                else:
                    dsb = dsbp.tile([128, 4, M], BF16, name="dsb", tag="dsb")
                    for i in range(n):
                        extract(t0 + i, dsb, i)
                # fold_l1 must precede colacc_group: the latter reduces
                # in-place into dsb slots 0..1
                if ring is None:
                    ff1 = ffp.tile([128, 8, 2048], BF16, name="ff1", tag="ff1")
                    fold_l1(dsb, n, ff1, 0)
                    ring = (ff1, t0, n)
                else:
                    ff1, t_start, fill = ring
                    fold_l1(dsb, n, ff1, fill)
                    fill += n
                    if fill >= 8:
                        fold_tail(ff1, fill, t_start)
                        ring = None
                    else:
                        ring = (ff1, t_start, fill)
                colacc_group(dsb, n)
            if ring is not None:
                ff1, t_start, fill = ring
                fold_tail(ff1, fill, t_start)

            # ---- tail: min over partitions of colacc via PE transpose ----
            colmins = persist.tile([128, NT], F32)
            for g in range(4):  # 4 groups of 8 [128,128] blocks
                pst = psp.tile([128, 1024], BF16, name="ps_tr", tag="ps_main")
                for k in range(8):
                    b = 8 * g + k
                    nc.tensor.matmul(
                        pst[:, 128 * k : 128 * (k + 1)],
                        colacc[:, 128 * b : 128 * (b + 1)],
                        ident16,
                        is_transpose=True,
                        start=True,
                        stop=True,
                    )
                nc.vector.tensor_reduce(
                    out=colmins[:, 8 * g : 8 * (g + 1)],
                    in_=pst.rearrange("p (k i) -> p k i", i=128),
                    axis=mybir.AxisListType.X,
                    op=mybir.AluOpType.min,
                )

            # ---- final sums (clamp the minima at 0 first) ----
            for mins in (rowmins, colmins):
                nc.vector.tensor_scalar(
                    out=mins, in0=mins, scalar1=0.0, scalar2=None,
                    op0=mybir.AluOpType.max,
                )
            rowsum = persist.tile([128, 1], F32)
            colsum = persist.tile([128, 1], F32)
            nc.vector.tensor_reduce(
                out=rowsum, in_=rowmins, axis=mybir.AxisListType.X,
                op=mybir.AluOpType.add,
            )
            nc.vector.tensor_reduce(
                out=colsum, in_=colmins, axis=mybir.AxisListType.X,
                op=mybir.AluOpType.add,
            )
            total = persist.tile([128, 1], F32)
            nc.vector.tensor_add(total, rowsum, colsum)
            ps_s = psp.tile([1, 1], F32, name="ps_s", tag="ps_main")
            ones = nc.const_aps.tensor(1.0, (128, 1))
            nc.tensor.matmul(ps_s, ones, total, start=True, stop=True)
            res_sb = persist.tile([1, 1], F32)
            nc.scalar.mul(res_sb, ps_s, 1.0 / float(N))
            nc.sync.dma_start(out=out_d.ap(), in_=res_sb)
            psp.release()

    _split_multi_waits(nc)
    return nc


# ---------------------------------------------------------------------------
# host-side strip construction (f32r == RNE at 11 mantissa bits, verified)


def _rne11(v: np.ndarray) -> np.ndarray:
    b = v.astype(np.float32).view(np.uint32).astype(np.uint64)
    shift = np.uint64(12)  # 23 - 11
    half = np.uint64(1 << 11)
    lsb = (b >> shift) & np.uint64(1)
    r = (b + half - np.uint64(1) + lsb) >> shift << shift
    return r.astype(np.uint32).view(np.float32)


def _strips(p: np.ndarray, g: np.ndarray):
    """Build zp [7, N], zg [7, M] fp32 strips for one batch element."""
    pr = _rne11(p)  # [N, 3] PE-rounded coords
    gr = _rne11(g)
    nP = (pr.astype(np.float64) ** 2).sum(1)
    nG = (gr.astype(np.float64) ** 2).sum(1)
    nPh = _rne11(nP.astype(np.float32))
    nPl = _rne11((nP - nPh.astype(np.float64)).astype(np.float32))
    nGh = _rne11(nG.astype(np.float32))
    nGl = _rne11((nG - nGh.astype(np.float64)).astype(np.float32))
    one = np.ones(N, dtype=np.float32)
    zp = np.stack([one, one, nPh, nPl, -2.0 * pr[:, 0], -2.0 * pr[:, 1],
                   -2.0 * pr[:, 2]])
    zg = np.stack([nGh, nGl, one, one, gr[:, 0], gr[:, 1], gr[:, 2]])
    return np.ascontiguousarray(zp), np.ascontiguousarray(zg)


_NC = None


def _get_nc():
    global _NC
    if _NC is None:
        _NC = build_nc()
    return _NC


def _ensure_ntff_hook():
    """Register the axon NTFF profiling hook if the container's antenv stub
    lacks axon_hooks (trace support; harmless to skip)."""
    import types

    try:
        import antenv
    except ImportError:
        return
    if not hasattr(antenv, "axon_hooks") or not hasattr(
        getattr(antenv, "axon_hooks", None), "get_axon_ntff_profile_hook"
    ):
        mod = types.ModuleType("antenv.axon_hooks")
        mod._h = None
        mod.set_axon_ntff_profile_hook = lambda h: setattr(mod, "_h", h)
        mod.get_axon_ntff_profile_hook = lambda: mod._h
        sys.modules["antenv.axon_hooks"] = mod
        antenv.axon_hooks = mod
    from antenv import axon_hooks

    if axon_hooks.get_axon_ntff_profile_hook() is None:
        try:
            from trn_agent_boot.trn_boot import _ntff_profile_via_ctypes

            hook = _ntff_profile_via_ctypes("/opt/axon/libaxon_pjrt.so")
            if hook is not None:
                axon_hooks.set_axon_ntff_profile_hook(hook)
        except Exception:
            pass


def kernel(pred_points: np.ndarray, gt_points: np.ndarray, _want_trace: bool = False):
    pred = np.ascontiguousarray(np.asarray(pred_points, dtype=np.float32))
    gt = np.ascontiguousarray(np.asarray(gt_points, dtype=np.float32))
    assert pred.shape == (B, N, D) and gt.shape == (B, M, D)

    in_maps = []
    for b in range(B):
        zp, zg = _strips(pred[b], gt[b])
        in_maps.append({"zp": zp, "zg": zg})

    nc = _get_nc()
    if _want_trace:
        _ensure_ntff_hook()
    res = run_bass_kernel_spmd(nc, in_maps, core_ids=list(range(B)), trace=_want_trace)
    per_core = np.array([r["out"][0, 0] for r in res.results], dtype=np.float64)
    loss = np.float32(per_core.mean())
    if _want_trace:
        return loss, res
    return loss


# revision 13
# speedup vs baseline: 1.0357x; 1.0357x over previous
"""Chamfer loss kernel for Trainium2 (8 NeuronCores, data-parallel over batch).

Problem: B=8, N=M=4096, D=3 fp32 point clouds.
  loss = mean_b mean_n min_m ||p_bn - g_bm||^2  +  mean_b mean_m min_n ||.||^2

Per-core structure (one batch element per core):
  - K=7 f32r matmuls produce distance tiles d[n, m] in PSUM (row pairing
    lhsT row k x rhs row k): k0,k1: 1*(|g|^2 hi/lo); k2,k3: (|p|^2 hi/lo)*1;
    k4-6: (-2 p_d)*g_d. The operand strips are fully precomputed on the HOST
    (f32r == round-to-nearest-even at 11 mantissa bits, verified on HW), so
    on-chip prep is just 2 HBM loads + 6 partition-shift DMAs replicating the
    strips to the 4 PE row-group bases (tile_position packing: 4 concurrent
    matmuls per [128, 2048] span).
  - Engine split (measured costs / span of [128,2048]):
      ACT: PSUM->SBUF bf16 extraction, 1.96us/span, 30 of 32 tiles.
      DVE: all reductions: rowmin via bf16 fold-trees batched over 8-tile
           rings (TT min at 2x mode, fold 4096->128, then one 1x
           tensor_reduce), and colacc via quad-batched TT min chains.
      Tiles 0-1 use the DVE fused tensor_scalar(max0, accum_out=min) path
      reading PSUM directly (extraction+rowmin in one 1x op) so DVE starts
      ~8us earlier while ACT warms up.
  - Tail: colacc [128, 4096] partition-min via PE transpose (bf16 identity)
    + 4 pipelined free-axis reduces; clamp minima at 0; final sums via
    ones-matmul; host averages the 8 per-core scalars.
"""

import sys

import numpy as np

sys.path.insert(0, "/opt/trn_rl_repo")

import bass_rust
import concourse.bass as bass
import concourse.mybir as mybir
from concourse.bass_utils import run_bass_kernel_spmd
from concourse.masks import make_identity
from concourse.tile import TileContext

B, N, M, D = 8, 4096, 4096, 3
NT = N // 128  # 32 n-tiles
K = 7
F32 = mybir.dt.float32
F32R = mybir.dt.float32r
BF16 = mybir.dt.bfloat16
BIG = 3.0e38

# ---------------------------------------------------------------------------
# walrus in this container rejects >1 sync-wait per instruction; spill the
# extras onto engine-matched NoOps placed immediately before the instruction.
_nop_counter = [0]


def _split_multi_waits(nc):
    for func in nc.m.functions:
        for bb in func.blocks:
            out = []
            dirty = False
            for inst in bb.instructions:
                si = inst.sync_info
                if si is not None and len(si.on_wait) > 1:
                    waits = list(si.on_wait)
                    for w in waits[:-1]:
                        _nop_counter[0] += 1
                        nop = mybir.InstNoOp(
                            name=f"I-waitsplit-{_nop_counter[0]}", ins=[], outs=[]
                        )
                        nop.engine = inst.engine
                        nop.sync_info = bass_rust.SyncInfo(on_wait=[w], on_update=[])
                        out.append(nop)
                    inst.sync_info = bass_rust.SyncInfo(
                        on_wait=[waits[-1]], on_update=list(si.on_update)
                    )
                    dirty = True
                out.append(inst)
            if dirty:
                bb.instructions = out
    return nc


# ---------------------------------------------------------------------------


def build_nc():
    nc = bass.Bass("TRN2")
    zp_d = nc.dram_tensor("zp", [K, N], F32, kind="ExternalInput")
    zg_d = nc.dram_tensor("zg", [K, M], F32, kind="ExternalInput")
    out_d = nc.dram_tensor("out", [1, 1], F32, kind="ExternalOutput")

    with TileContext(nc) as tc:
        with (
            tc.tile_pool(name="persist", bufs=1) as persist,
            tc.tile_pool(name="dsbp", bufs=2) as dsbp,
            tc.tile_pool(name="ffp", bufs=1) as ffp,
        ):
            # --- prep: load strips fp32 via fast HWDGE, cast to f32r on
            # DVE (zp) + ACT (zg) in parallel, replicate to the 4 PE
            # row-group bases via sync/scalar HWDGE queues (gpsimd SWDGE is
            # slow and its drains can block the PE). ---
            warm = persist.tile([1, 8], F32)
            nc.vector.memset(warm, 0.0)
            nc.scalar.copy(out=warm, in_=warm)  # hoist ACT_TABLE_LOAD
            zp_f = persist.tile([K, N], F32)
            zg_f = persist.tile([K, M], F32)
            nc.sync.dma_start(out=zp_f, in_=zp_d.ap())
            nc.scalar.dma_start(out=zg_f, in_=zg_d.ap())
            zp_pk = persist.tile([96 + K, N], F32R)
            zg_pk = persist.tile([96 + K, M], F32R)
            nc.vector.tensor_copy(zp_pk[0:K, :], zp_f)
            nc.scalar.copy(zg_pk[0:K, :], zg_f)
            qs = [nc.sync, nc.scalar]
            for g in range(1, 4):
                qs[g % 2].dma_start(
                    out=zp_pk[32 * g : 32 * g + K, :], in_=zp_pk[0:K, :]
                )
                qs[(g + 1) % 2].dma_start(
                    out=zg_pk[32 * g : 32 * g + K, :], in_=zg_pk[0:K, :]
                )
            # identity for tail transposes (off critical path)
            ident16 = persist.tile([128, 128], BF16)
            nc.vector.memset(ident16, 0.0)
            make_identity(nc, ident16, nomemset=True)

            colacc = persist.tile([128, M], BF16)
            rowmins = persist.tile([128, NT], F32)
            rowspan = persist.tile([128, 4], F32)

            psp = tc.alloc_tile_pool(name="psum_all", bufs=2, space="PSUM")

            def mm_span(t, h):
                """One [128, 2048] PSUM span for n-tile t, half h. Tiles
                t < 6 use row-group-0 only (sequential MMs) so they can
                start before the strip replicas land; later tiles use the
                4-way row-group packing (concurrent MMs)."""
                ps = psp.tile([128, 2048], F32, name="ps_main", tag="ps_main")
                for g in range(4):
                    col0 = 2048 * h + 512 * g
                    if t < 6:
                        nc.tensor.matmul(
                            ps[:, 512 * g : 512 * (g + 1)],
                            zp_pk[0:K, 128 * t : 128 * (t + 1)],
                            zg_pk[0:K, col0 : col0 + 512],
                            start=True,
                            stop=True,
                        )
                    else:
                        nc.tensor.matmul(
                            ps[:, 512 * g : 512 * (g + 1)],
                            zp_pk[32 * g : 32 * g + K, 128 * t : 128 * (t + 1)],
                            zg_pk[32 * g : 32 * g + K, col0 : col0 + 512],
                            start=True,
                            stop=True,
                            tile_position=(32 * g, 0),
                        )
                return ps

            def extract(t, dsb, slot):
                for h in range(2):
                    ps = mm_span(t, h)
                    nc.scalar.copy(
                        out=dsb[:, slot, 2048 * h : 2048 * (h + 1)], in_=ps
                    )

            # --- tiles 0, 1: DVE fused extraction (tensor_scalar from PSUM:
            # out = relu(d) -> bf16, accum_out = rowmin of span), span-
            # interleaved with ACT extraction of tiles 2, 3 so both engines
            # start working ~immediately. ---
            dsb23 = dsbp.tile([128, 4, M], BF16, name="dsb", tag="dsb")
            d1 = dsb23[:, 3, :]
            for t in range(2):
                for h in range(2):
                    ps = mm_span(t, h)
                    dest = colacc if t == 0 else d1
                    nc.vector.tensor_scalar(
                        out=dest[:, 2048 * h : 2048 * (h + 1)],
                        in0=ps,
                        scalar1=0.0,
                        scalar2=BIG,
                        op0=mybir.AluOpType.max,
                        op1=mybir.AluOpType.min,
                        accum_out=rowspan[:, 2 * t + h : 2 * t + h + 1],
                    )
                    ps2 = mm_span(t + 2, h)
                    nc.scalar.copy(
                        out=dsb23[:, t, 2048 * h : 2048 * (h + 1)], in_=ps2
                    )
            nc.vector.tensor_tensor(
                out=colacc, in0=d1, in1=colacc, op=mybir.AluOpType.min
            )
            # rowmins[:, 0:2] = pairwise min of the 4 span minima
            nc.vector.tensor_tensor(
                out=rowmins[:, 0:2],
                in0=rowspan.rearrange("p (t h) -> p t h", h=2)[:, :, 0],
                in1=rowspan.rearrange("p (t h) -> p t h", h=2)[:, :, 1],
                op=mybir.AluOpType.min,
            )

            def colacc_group(dsb, n):
                # elementwise-min of n extracted tiles into colacc
                if n == 4:
                    nc.vector.tensor_tensor(
                        out=dsb[:, 0:2, :], in0=dsb[:, 0:2, :],
                        in1=dsb[:, 2:4, :], op=mybir.AluOpType.min,
                    )
                r = dsb[:, 0, :]
                nc.vector.tensor_tensor(
                    out=r, in0=dsb[:, 0, :], in1=dsb[:, 1, :],
                    op=mybir.AluOpType.min,
                )
                nc.vector.tensor_tensor(
                    out=colacc, in0=r, in1=colacc, op=mybir.AluOpType.min
                )

            def fold_l1(dsb, n, ff1, fill):
                # level-1 fold of n tiles into ff1[:, fill : fill+n, :]
                v = dsb.rearrange("p i (l m) -> p i l m", l=2)[:, 0:n]
                nc.vector.tensor_tensor(
                    out=ff1[:, fill : fill + n, :],
                    in0=v[:, :, 0, :],
                    in1=v[:, :, 1, :],
                    op=mybir.AluOpType.min,
                )

            def fold_tail(ff1, n, t0):
                # fold n packed tiles 2048 -> 128, then one batched reduce
                f = ff1
                w = 2048
                for name in ("ff2", "ff3", "ff4", "ff5"):
                    nf = ffp.tile([128, 8, w // 2], BF16, name=name, tag=name)
                    v = f.rearrange("p i (l m) -> p i l m", l=2)[:, 0:n]
                    nc.vector.tensor_tensor(
                        out=nf[:, 0:n, :], in0=v[:, :, 0, :], in1=v[:, :, 1, :],
                        op=mybir.AluOpType.min,
                    )
                    f = nf
                    w //= 2
                nc.vector.tensor_reduce(
                    out=rowmins[:, t0 : t0 + n],
                    in_=f[:, 0:n, :],
                    axis=mybir.AxisListType.X,
                    op=mybir.AluOpType.min,
                )

            # group plan over tiles 2..31: pairs {2,3} {4,5}, quads {6-9}..
            # {26-29}, pair {30,31}. Early pairs keep DVE fed while ACT
            # builds its lead; the fold ring accumulates L1 folds of
            # consecutive tiles and flushes a batched fold-tail at 8.
            groups = [(2, 2), (4, 2)] + [(6 + 4 * i, 4) for i in range(6)] + [(30, 2)]
            ring = None  # (ff1, t_start, fill)
            for gi, (t0, n) in enumerate(groups):
                if gi == 0:
                    dsb = dsb23  # already extracted (interleaved above)
                else:
                    dsb = dsbp.tile([128, 4, M], BF16, name="dsb", tag="dsb")
                    for i in range(n):
                        extract(t0 + i, dsb, i)
                # fold_l1 must precede colacc_group: the latter reduces
                # in-place into dsb slots 0..1
                flush = None
                if ring is None:
                    ff1 = ffp.tile([128, 8, 2048], BF16, name="ff1", tag="ff1")
                    fold_l1(dsb, n, ff1, 0)
                    ring = (ff1, t0, n)
                else:
                    ff1, t_start, fill = ring
                    fold_l1(dsb, n, ff1, fill)
                    fill += n
                    if fill >= 8:
                        flush = (ff1, fill, t_start)
                        ring = None
                    else:
                        ring = (ff1, t_start, fill)
                colacc_group(dsb, n)
                if flush is not None:
                    fold_tail(flush[0], flush[1], flush[2])
            if ring is not None:
                ff1, t_start, fill = ring
                fold_tail(ff1, fill, t_start)

            # ---- tail: min over partitions of colacc via PE transpose ----
            colmins = persist.tile([128, NT], F32)
            for g in range(4):  # 4 groups of 8 [128,128] blocks
                pst = psp.tile([128, 1024], BF16, name="ps_tr", tag="ps_main")
                for k in range(8):
                    b = 8 * g + k
                    nc.tensor.matmul(
                        pst[:, 128 * k : 128 * (k + 1)],
                        colacc[:, 128 * b : 128 * (b + 1)],
                        ident16,
                        is_transpose=True,
                        start=True,
                        stop=True,
                    )
                nc.vector.tensor_reduce(
                    out=colmins[:, 8 * g : 8 * (g + 1)],
                    in_=pst.rearrange("p (k i) -> p k i", i=128),
                    axis=mybir.AxisListType.X,
                    op=mybir.AluOpType.min,
                )

            # ---- final sums (clamp the minima at 0 first) ----
            for mins in (rowmins, colmins):
                nc.vector.tensor_scalar(
                    out=mins, in0=mins, scalar1=0.0, scalar2=None,
                    op0=mybir.AluOpType.max,
                )
            rowsum = persist.tile([128, 1], F32)
            colsum = persist.tile([128, 1], F32)
            nc.vector.tensor_reduce(
                out=rowsum, in_=rowmins, axis=mybir.AxisListType.X,
                op=mybir.AluOpType.add,
            )
            nc.vector.tensor_reduce(
                out=colsum, in_=colmins, axis=mybir.AxisListType.X,
                op=mybir.AluOpType.add,
            )
            total = persist.tile([128, 1], F32)
            nc.vector.tensor_add(total, rowsum, colsum)
            ps_s = psp.tile([1, 1], F32, name="ps_s", tag="ps_main")
            ones = nc.const_aps.tensor(1.0, (128, 1))
            nc.tensor.matmul(ps_s, ones, total, start=True, stop=True)
            res_sb = persist.tile([1, 1], F32)
            nc.scalar.mul(res_sb, ps_s, 1.0 / float(N))
            nc.sync.dma_start(out=out_d.ap(), in_=res_sb)
            psp.release()

    _split_multi_waits(nc)
    return nc


# ---------------------------------------------------------------------------
# host-side strip construction (f32r == RNE at 11 mantissa bits, verified)


def _rne11(v: np.ndarray) -> np.ndarray:
    b = v.astype(np.float32).view(np.uint32).astype(np.uint64)
    shift = np.uint64(12)  # 23 - 11
    half = np.uint64(1 << 11)
    lsb = (b >> shift) & np.uint64(1)
    r = (b + half - np.uint64(1) + lsb) >> shift << shift
    return r.astype(np.uint32).view(np.float32)


def _strips(p: np.ndarray, g: np.ndarray):
    """Build zp [7, N], zg [7, M] fp32 strips for one batch element."""
    pr = _rne11(p)  # [N, 3] PE-rounded coords
    gr = _rne11(g)
    nP = (pr.astype(np.float64) ** 2).sum(1)
    nG = (gr.astype(np.float64) ** 2).sum(1)
    nPh = _rne11(nP.astype(np.float32))
    nPl = _rne11((nP - nPh.astype(np.float64)).astype(np.float32))
    nGh = _rne11(nG.astype(np.float32))
    nGl = _rne11((nG - nGh.astype(np.float64)).astype(np.float32))
    one = np.ones(N, dtype=np.float32)
    zp = np.stack([one, one, nPh, nPl, -2.0 * pr[:, 0], -2.0 * pr[:, 1],
                   -2.0 * pr[:, 2]])
    zg = np.stack([nGh, nGl, one, one, gr[:, 0], gr[:, 1], gr[:, 2]])
    return np.ascontiguousarray(zp), np.ascontiguousarray(zg)


_NC = None


def _get_nc():
    global _NC
    if _NC is None:
        _NC = build_nc()
    return _NC


def _ensure_ntff_hook():
    """Register the axon NTFF profiling hook if the container's antenv stub
    lacks axon_hooks (trace support; harmless to skip)."""
    import types

    try:
        import antenv
    except ImportError:
        return
    if not hasattr(antenv, "axon_hooks") or not hasattr(
        getattr(antenv, "axon_hooks", None), "get_axon_ntff_profile_hook"
    ):
        mod = types.ModuleType("antenv.axon_hooks")
        mod._h = None
        mod.set_axon_ntff_profile_hook = lambda h: setattr(mod, "_h", h)
        mod.get_axon_ntff_profile_hook = lambda: mod._h
        sys.modules["antenv.axon_hooks"] = mod
        antenv.axon_hooks = mod
    from antenv import axon_hooks

    if axon_hooks.get_axon_ntff_profile_hook() is None:
        try:
            from trn_agent_boot.trn_boot import _ntff_profile_via_ctypes

            hook = _ntff_profile_via_ctypes("/opt/axon/libaxon_pjrt.so")
            if hook is not None:
                axon_hooks.set_axon_ntff_profile_hook(hook)
        except Exception:
            pass


def kernel(pred_points: np.ndarray, gt_points: np.ndarray, _want_trace: bool = False):
    pred = np.ascontiguousarray(np.asarray(pred_points, dtype=np.float32))
    gt = np.ascontiguousarray(np.asarray(gt_points, dtype=np.float32))
    assert pred.shape == (B, N, D) and gt.shape == (B, M, D)

    in_maps = []
    for b in range(B):
        zp, zg = _strips(pred[b], gt[b])
        in_maps.append({"zp": zp, "zg": zg})

    nc = _get_nc()
    if _want_trace:
        _ensure_ntff_hook()
    res = run_bass_kernel_spmd(nc, in_maps, core_ids=list(range(B)), trace=_want_trace)
    per_core = np.array([r["out"][0, 0] for r in res.results], dtype=np.float64)
    loss = np.float32(per_core.mean())
    if _want_trace:
        return loss, res
    return loss


# revision 14
# speedup vs baseline: 1.0559x; 1.0194x over previous
"""Chamfer loss kernel for Trainium2 (8 NeuronCores, data-parallel over batch).

Problem: B=8, N=M=4096, D=3 fp32 point clouds.
  loss = mean_b mean_n min_m ||p_bn - g_bm||^2  +  mean_b mean_m min_n ||.||^2

Per-core structure (one batch element per core):
  - K=7 f32r matmuls produce distance tiles d[n, m] in PSUM (row pairing
    lhsT row k x rhs row k): k0,k1: 1*(|g|^2 hi/lo); k2,k3: (|p|^2 hi/lo)*1;
    k4-6: (-2 p_d)*g_d. The operand strips are fully precomputed on the HOST
    (f32r == round-to-nearest-even at 11 mantissa bits, verified on HW), so
    on-chip prep is just 2 HBM loads + 6 partition-shift DMAs replicating the
    strips to the 4 PE row-group bases (tile_position packing: 4 concurrent
    matmuls per [128, 2048] span).
  - Engine split (measured costs / span of [128,2048]):
      ACT: PSUM->SBUF bf16 extraction, 1.96us/span, 30 of 32 tiles.
      DVE: all reductions: rowmin via bf16 fold-trees batched over 8-tile
           rings (TT min at 2x mode, fold 4096->128, then one 1x
           tensor_reduce), and colacc via quad-batched TT min chains.
      Tiles 0-1 use the DVE fused tensor_scalar(max0, accum_out=min) path
      reading PSUM directly (extraction+rowmin in one 1x op) so DVE starts
      ~8us earlier while ACT warms up.
  - Tail: colacc [128, 4096] partition-min via PE transpose (bf16 identity)
    + 4 pipelined free-axis reduces; clamp minima at 0; final sums via
    ones-matmul; host averages the 8 per-core scalars.
"""

import sys

import numpy as np

sys.path.insert(0, "/opt/trn_rl_repo")

import bass_rust
import concourse.bass as bass
import concourse.mybir as mybir
from concourse.bass_utils import run_bass_kernel_spmd
from concourse.masks import make_identity
from concourse.tile import TileContext

B, N, M, D = 8, 4096, 4096, 3
NT = N // 128  # 32 n-tiles
K = 7
F32 = mybir.dt.float32
F32R = mybir.dt.float32r
BF16 = mybir.dt.bfloat16
BIG = 3.0e38

# ---------------------------------------------------------------------------
# walrus in this container rejects >1 sync-wait per instruction; spill the
# extras onto engine-matched NoOps placed immediately before the instruction.
_nop_counter = [0]


def _split_multi_waits(nc):
    for func in nc.m.functions:
        for bb in func.blocks:
            out = []
            dirty = False
            for inst in bb.instructions:
                si = inst.sync_info
                if si is not None and len(si.on_wait) > 1:
                    waits = list(si.on_wait)
                    for w in waits[:-1]:
                        _nop_counter[0] += 1
                        nop = mybir.InstNoOp(
                            name=f"I-waitsplit-{_nop_counter[0]}", ins=[], outs=[]
                        )
                        nop.engine = inst.engine
                        nop.sync_info = bass_rust.SyncInfo(on_wait=[w], on_update=[])
                        out.append(nop)
                    inst.sync_info = bass_rust.SyncInfo(
                        on_wait=[waits[-1]], on_update=list(si.on_update)
                    )
                    dirty = True
                out.append(inst)
            if dirty:
                bb.instructions = out
    return nc


# ---------------------------------------------------------------------------


def build_nc():
    nc = bass.Bass("TRN2")
    zp_d = nc.dram_tensor("zp", [K, N], F32, kind="ExternalInput")
    zg_d = nc.dram_tensor("zg", [K, M], F32, kind="ExternalInput")
    out_d = nc.dram_tensor("out", [1, 1], F32, kind="ExternalOutput")

    with TileContext(nc) as tc:
        with (
            tc.tile_pool(name="persist", bufs=1) as persist,
            tc.tile_pool(name="dsbp", bufs=2) as dsbp,
            tc.tile_pool(name="ffp", bufs=1) as ffp,
        ):
            # --- prep: load strips fp32 via fast HWDGE, cast to f32r on
            # DVE (zp) + ACT (zg) in parallel, replicate to the 4 PE
            # row-group bases via sync/scalar HWDGE queues (gpsimd SWDGE is
            # slow and its drains can block the PE). ---
            warm = persist.tile([1, 8], F32)
            nc.vector.memset(warm, 0.0)
            nc.scalar.copy(out=warm, in_=warm)  # hoist ACT_TABLE_LOAD
            zp_f = persist.tile([K, N], F32)
            zg_f = persist.tile([K, M], F32)
            nc.sync.dma_start(out=zp_f, in_=zp_d.ap())
            nc.scalar.dma_start(out=zg_f, in_=zg_d.ap())
            zp_pk = persist.tile([96 + K, N], F32R)
            zg_pk = persist.tile([96 + K, M], F32R)
            nc.vector.tensor_copy(zp_pk[0:K, :], zp_f)
            nc.scalar.copy(zg_pk[0:K, :], zg_f)
            qs = [nc.sync, nc.scalar]
            for g in range(1, 4):
                qs[g % 2].dma_start(
                    out=zp_pk[32 * g : 32 * g + K, :], in_=zp_pk[0:K, :]
                )
                qs[(g + 1) % 2].dma_start(
                    out=zg_pk[32 * g : 32 * g + K, :], in_=zg_pk[0:K, :]
                )
            # identity for tail transposes (off critical path)
            ident16 = persist.tile([128, 128], BF16)
            nc.vector.memset(ident16, 0.0)
            make_identity(nc, ident16, nomemset=True)

            colacc = persist.tile([128, M], BF16)
            rowmins = persist.tile([128, NT], F32)
            rowspan = persist.tile([128, 4], F32)

            psp = tc.alloc_tile_pool(name="psum_all", bufs=2, space="PSUM")

            def mm_span(t, h):
                """One [128, 2048] PSUM span for n-tile t, half h. Tiles
                t < 6 use row-group-0 only (sequential MMs) so they can
                start before the strip replicas land; later tiles use the
                4-way row-group packing (concurrent MMs)."""
                ps = psp.tile([128, 2048], F32, name="ps_main", tag="ps_main")
                for g in range(4):
                    col0 = 2048 * h + 512 * g
                    if t < 4:
                        nc.tensor.matmul(
                            ps[:, 512 * g : 512 * (g + 1)],
                            zp_pk[0:K, 128 * t : 128 * (t + 1)],
                            zg_pk[0:K, col0 : col0 + 512],
                            start=True,
                            stop=True,
                        )
                    else:
                        nc.tensor.matmul(
                            ps[:, 512 * g : 512 * (g + 1)],
                            zp_pk[32 * g : 32 * g + K, 128 * t : 128 * (t + 1)],
                            zg_pk[32 * g : 32 * g + K, col0 : col0 + 512],
                            start=True,
                            stop=True,
                            tile_position=(32 * g, 0),
                        )
                return ps

            def extract(t, dsb, slot):
                for h in range(2):
                    ps = mm_span(t, h)
                    nc.scalar.copy(
                        out=dsb[:, slot, 2048 * h : 2048 * (h + 1)], in_=ps
                    )

            # --- tiles 0, 1: DVE fused extraction (tensor_scalar from PSUM:
            # out = relu(d) -> bf16, accum_out = rowmin of span), span-
            # interleaved with ACT extraction of tiles 2, 3 so both engines
            # start working ~immediately. ---
            dsb23 = dsbp.tile([128, 4, M], BF16, name="dsb", tag="dsb")
            d1 = dsb23[:, 3, :]
            for t in range(2):
                for h in range(2):
                    ps = mm_span(t, h)
                    dest = colacc if t == 0 else d1
                    nc.vector.tensor_scalar(
                        out=dest[:, 2048 * h : 2048 * (h + 1)],
                        in0=ps,
                        scalar1=0.0,
                        scalar2=BIG,
                        op0=mybir.AluOpType.max,
                        op1=mybir.AluOpType.min,
                        accum_out=rowspan[:, 2 * t + h : 2 * t + h + 1],
                    )
                    ps2 = mm_span(t + 2, h)
                    nc.scalar.copy(
                        out=dsb23[:, t, 2048 * h : 2048 * (h + 1)], in_=ps2
                    )
            nc.vector.tensor_tensor(
                out=colacc, in0=d1, in1=colacc, op=mybir.AluOpType.min
            )
            # rowmins[:, 0:2] = pairwise min of the 4 span minima
            nc.vector.tensor_tensor(
                out=rowmins[:, 0:2],
                in0=rowspan.rearrange("p (t h) -> p t h", h=2)[:, :, 0],
                in1=rowspan.rearrange("p (t h) -> p t h", h=2)[:, :, 1],
                op=mybir.AluOpType.min,
            )

            def colacc_group(dsb, n):
                # elementwise-min of n extracted tiles into colacc
                if n == 4:
                    nc.vector.tensor_tensor(
                        out=dsb[:, 0:2, :], in0=dsb[:, 0:2, :],
                        in1=dsb[:, 2:4, :], op=mybir.AluOpType.min,
                    )
                r = dsb[:, 0, :]
                nc.vector.tensor_tensor(
                    out=r, in0=dsb[:, 0, :], in1=dsb[:, 1, :],
                    op=mybir.AluOpType.min,
                )
                nc.vector.tensor_tensor(
                    out=colacc, in0=r, in1=colacc, op=mybir.AluOpType.min
                )

            def fold_l1(dsb, n, ff1, fill):
                # level-1 fold of n tiles into ff1[:, fill : fill+n, :]
                v = dsb.rearrange("p i (l m) -> p i l m", l=2)[:, 0:n]
                nc.vector.tensor_tensor(
                    out=ff1[:, fill : fill + n, :],
                    in0=v[:, :, 0, :],
                    in1=v[:, :, 1, :],
                    op=mybir.AluOpType.min,
                )

            def fold_tail(ff1, n, t0):
                # fold n packed tiles 2048 -> 128, then one batched reduce
                f = ff1
                w = 2048
                for name in ("ff2", "ff3", "ff4", "ff5"):
                    nf = ffp.tile([128, 8, w // 2], BF16, name=name, tag=name)
                    v = f.rearrange("p i (l m) -> p i l m", l=2)[:, 0:n]
                    nc.vector.tensor_tensor(
                        out=nf[:, 0:n, :], in0=v[:, :, 0, :], in1=v[:, :, 1, :],
                        op=mybir.AluOpType.min,
                    )
                    f = nf
                    w //= 2
                nc.vector.tensor_reduce(
                    out=rowmins[:, t0 : t0 + n],
                    in_=f[:, 0:n, :],
                    axis=mybir.AxisListType.X,
                    op=mybir.AluOpType.min,
                )

            # group plan over tiles 2..31: pairs {2,3} {4,5}, quads {6-9}..
            # {26-29}, pair {30,31}. Early pairs keep DVE fed while ACT
            # builds its lead; the fold ring accumulates L1 folds of
            # consecutive tiles and flushes a batched fold-tail at 8.
            groups = [(2 + 2 * i, 2) for i in range(15)]
            ring = None  # (ff1, t_start, fill)
            for gi, (t0, n) in enumerate(groups):
                if gi == 0:
                    dsb = dsb23  # already extracted (interleaved above)
                else:
                    dsb = dsbp.tile([128, 4, M], BF16, name="dsb", tag="dsb")
                    for i in range(n):
                        extract(t0 + i, dsb, i)
                # fold_l1 must precede colacc_group: the latter reduces
                # in-place into dsb slots 0..1
                flush = None
                if ring is None:
                    ff1 = ffp.tile([128, 8, 2048], BF16, name="ff1", tag="ff1")
                    fold_l1(dsb, n, ff1, 0)
                    ring = (ff1, t0, n)
                else:
                    ff1, t_start, fill = ring
                    fold_l1(dsb, n, ff1, fill)
                    fill += n
                    if fill >= 8:
                        flush = (ff1, fill, t_start)
                        ring = None
                    else:
                        ring = (ff1, t_start, fill)
                colacc_group(dsb, n)
                if flush is not None:
                    fold_tail(flush[0], flush[1], flush[2])
            if ring is not None:
                ff1, t_start, fill = ring
                fold_tail(ff1, fill, t_start)

            # ---- tail: min over partitions of colacc via PE transpose ----
            colmins = persist.tile([128, NT], F32)
            for g in range(4):  # 4 groups of 8 [128,128] blocks
                pst = psp.tile([128, 1024], BF16, name="ps_tr", tag="ps_main")
                for k in range(8):
                    b = 8 * g + k
                    nc.tensor.matmul(
                        pst[:, 128 * k : 128 * (k + 1)],
                        colacc[:, 128 * b : 128 * (b + 1)],
                        ident16,
                        is_transpose=True,
                        start=True,
                        stop=True,
                    )
                nc.vector.tensor_reduce(
                    out=colmins[:, 8 * g : 8 * (g + 1)],
                    in_=pst.rearrange("p (k i) -> p k i", i=128),
                    axis=mybir.AxisListType.X,
                    op=mybir.AluOpType.min,
                )

            # ---- final sums (clamp the minima at 0 first) ----
            for mins in (rowmins, colmins):
                nc.vector.tensor_scalar(
                    out=mins, in0=mins, scalar1=0.0, scalar2=None,
                    op0=mybir.AluOpType.max,
                )
            rowsum = persist.tile([128, 1], F32)
            colsum = persist.tile([128, 1], F32)
            nc.vector.tensor_reduce(
                out=rowsum, in_=rowmins, axis=mybir.AxisListType.X,
                op=mybir.AluOpType.add,
            )
            nc.vector.tensor_reduce(
                out=colsum, in_=colmins, axis=mybir.AxisListType.X,
                op=mybir.AluOpType.add,
            )
            total = persist.tile([128, 1], F32)
            nc.vector.tensor_add(total, rowsum, colsum)
            ps_s = psp.tile([1, 1], F32, name="ps_s", tag="ps_main")
            ones = nc.const_aps.tensor(1.0, (128, 1))
            nc.tensor.matmul(ps_s, ones, total, start=True, stop=True)
            res_sb = persist.tile([1, 1], F32)
            nc.scalar.mul(res_sb, ps_s, 1.0 / float(N))
            nc.sync.dma_start(out=out_d.ap(), in_=res_sb)
            psp.release()

    _split_multi_waits(nc)
    return nc


# ---------------------------------------------------------------------------
# host-side strip construction (f32r == RNE at 11 mantissa bits, verified)


def _rne11(v: np.ndarray) -> np.ndarray:
    b = v.astype(np.float32).view(np.uint32).astype(np.uint64)
    shift = np.uint64(12)  # 23 - 11
    half = np.uint64(1 << 11)
    lsb = (b >> shift) & np.uint64(1)
    r = (b + half - np.uint64(1) + lsb) >> shift << shift
    return r.astype(np.uint32).view(np.float32)


def _strips(p: np.ndarray, g: np.ndarray):
    """Build zp [7, N], zg [7, M] fp32 strips for one batch element."""
    pr = _rne11(p)  # [N, 3] PE-rounded coords
    gr = _rne11(g)
    nP = (pr.astype(np.float64) ** 2).sum(1)
    nG = (gr.astype(np.float64) ** 2).sum(1)
    nPh = _rne11(nP.astype(np.float32))
    nPl = _rne11((nP - nPh.astype(np.float64)).astype(np.float32))
    nGh = _rne11(nG.astype(np.float32))
    nGl = _rne11((nG - nGh.astype(np.float64)).astype(np.float32))
    one = np.ones(N, dtype=np.float32)
    zp = np.stack([one, one, nPh, nPl, -2.0 * pr[:, 0], -2.0 * pr[:, 1],
                   -2.0 * pr[:, 2]])
    zg = np.stack([nGh, nGl, one, one, gr[:, 0], gr[:, 1], gr[:, 2]])
    return np.ascontiguousarray(zp), np.ascontiguousarray(zg)


_NC = None


def _get_nc():
    global _NC
    if _NC is None:
        _NC = build_nc()
    return _NC


def _ensure_ntff_hook():
    """Register the axon NTFF profiling hook if the container's antenv stub
    lacks axon_hooks (trace support; harmless to skip)."""
    import types

    try:
        import antenv
    except ImportError:
        return
    if not hasattr(antenv, "axon_hooks") or not hasattr(
        getattr(antenv, "axon_hooks", None), "get_axon_ntff_profile_hook"
    ):
        mod = types.ModuleType("antenv.axon_hooks")
        mod._h = None
        mod.set_axon_ntff_profile_hook = lambda h: setattr(mod, "_h", h)
        mod.get_axon_ntff_profile_hook = lambda: mod._h
        sys.modules["antenv.axon_hooks"] = mod
        antenv.axon_hooks = mod
    from antenv import axon_hooks

    if axon_hooks.get_axon_ntff_profile_hook() is None:
        try:
            from trn_agent_boot.trn_boot import _ntff_profile_via_ctypes

            hook = _ntff_profile_via_ctypes("/opt/axon/libaxon_pjrt.so")
            if hook is not None:
                axon_hooks.set_axon_ntff_profile_hook(hook)
        except Exception:
            pass


def kernel(pred_points: np.ndarray, gt_points: np.ndarray, _want_trace: bool = False):
    pred = np.ascontiguousarray(np.asarray(pred_points, dtype=np.float32))
    gt = np.ascontiguousarray(np.asarray(gt_points, dtype=np.float32))
    assert pred.shape == (B, N, D) and gt.shape == (B, M, D)

    in_maps = []
    for b in range(B):
        zp, zg = _strips(pred[b], gt[b])
        in_maps.append({"zp": zp, "zg": zg})

    nc = _get_nc()
    if _want_trace:
        _ensure_ntff_hook()
    res = run_bass_kernel_spmd(nc, in_maps, core_ids=list(range(B)), trace=_want_trace)
    per_core = np.array([r["out"][0, 0] for r in res.results], dtype=np.float64)
    loss = np.float32(per_core.mean())
    if _want_trace:
        return loss, res
    return loss
